# revision 26
# baseline (speedup 1.0000x reference)
"""Trainium2 Bass kernel for nn_CPLinear (CP-decomposed QKV projection with RoPE).

Computes, for x:(2,4096,2048) and CP-factor weights:
    A_t = x @ W_A_t  (per-token head coefficients),  B_t = x @ W_B_t (shared bases)
    q = einsum('bshr,bsrd->bshd', A_q, rope(B_q)) / 12
    k = A_k * rope(B_k)   (rank-1)
    v = A_v * B_v         (rank-1)

Strategy (8 cores, data-parallel over the 8192 tokens, 1024 tokens/core):
  - All 6 projections fused into one [2048 x 2016] bf16 matmul (PE), with the
    1/12 scale and (h,r)->(r,h) reorder folded into W_A_q host-side.
  - W/x are DMAed k-chunk-interleaved; tiles 0+1 run a k-outer PSUM-resident
    warmup so the PE starts ~2us into the load instead of after it.
  - PSUM is one shared 8x[128,512] pool: a produce takes 4 banks for the
    fused projection, a consume takes 4 for the block-diagonal q matmuls;
    strict p/c alternation keeps the rotation conflict-free.
  - RoPE applied to B_q/B_k with bf16 tensor_tensor ops (cos/sin tables are
    host-precomputed per-token inputs, replicated x12 along r).
  - The per-token rank-12 contraction for q runs on the PE as a block-diagonal
    matmul: 8 tokens/matmul, K=96=(8 tokens x 12 r), M=128=(8 tokens x 16 h),
    N=128=d. Operands are built by a DRAM bounce + per-slot scatter DMAs
    spread over 4 engines.
  - k/v are per-partition-scalar broadcasts (DVE tensor_tensor).
  - Outputs are written bf16 in on-chip layout; host reorders q and widens.
"""

import sys

for _p in ("/opt/trn_rl_repo",):
    if _p not in sys.path:
        sys.path.insert(0, _p)

import numpy as np
import ml_dtypes

BF16 = ml_dtypes.bfloat16

SH = 1024          # tokens per core
H = 2048           # hidden
KT = H // 128      # 16 k-tiles
NT = SH // 128     # 8 token tiles per core
NOUT = 2016        # fused projection output width
NH, HD, RQ = 16, 128, 12

_CACHE = {}


def make_nc():
    import concourse.bacc as bacc
    from concourse import mybir

    dt = mybir.dt

    nc = bacc.Bacc(
        "TRN2",
        target_bir_lowering=False,
        debug=False,
        enable_asserts=False,
        num_devices=8,
    )

    x_d = nc.dram_tensor("x", (H, SH), dt.bfloat16, kind="ExternalInput")  # pre-transposed host-side
    w_d = nc.dram_tensor("w", (KT, 128, NOUT), dt.bfloat16, kind="ExternalInput")
    # cos2 = [cos, cos], sinsw = [+sin, -sin] (rotate-half folded into sign)
    cos_d = nc.dram_tensor("cos2", (SH, 128), dt.bfloat16, kind="ExternalInput")
    sin_d = nc.dram_tensor("sinsw", (SH, 128), dt.bfloat16, kind="ExternalInput")
    # q in on-chip layout: row (tile,t,h) = tile*128 + t*16 + h, col g*128+d
    q_d = nc.dram_tensor("q", (SH, NH * HD), dt.bfloat16, kind="ExternalOutput")
    k_d = nc.dram_tensor("k", (SH, NH * HD), dt.bfloat16, kind="ExternalOutput")
    v_d = nc.dram_tensor("v", (SH, NH * HD), dt.bfloat16, kind="ExternalOutput")
    return nc, (x_d, w_d, cos_d, sin_d, q_d, k_d, v_d)


def build_body(nc, tc, tensors):
    from contextlib import ExitStack

    from concourse import mybir

    dt = mybir.dt
    x_d, w_d, cos_d, sin_d, q_d, k_d, v_d = tensors

    with ExitStack() as ctx:
        P = ctx.enter_context
        const_pool = P(tc.tile_pool(name="const", bufs=1))
        w_sb = const_pool.tile([128, KT * NOUT], dt.bfloat16, tag="w_sb")
        cos_sb = const_pool.tile([128, NT * 128], dt.bfloat16, tag="cos_sb")
        sin_sb = const_pool.tile([128, NT * 128], dt.bfloat16, tag="sin_sb")
        xT = const_pool.tile([128, KT * SH], dt.bfloat16, tag="xT")
        # combined block-diagonal operand holders: cols 0:2048 = stacked roped
        # B_q (rhs), cols 2048:4096 = block-diag A' (lhsT), per 8-token slot.
        bd_bufs = [
            const_pool.tile([128, 4096], dt.bfloat16, tag=f"bd{i}", name=f"bd{i}")
            for i in range(3)
        ]

        # constant loads: cos/sin first (tiny), then W/x k-chunk interleaved
        # so the k-outer warmup on tiles 0/1 can start after the first chunk.
        w_v = w_sb[:].rearrange("p (k n) -> p k n", k=KT)
        wd_v = w_d[:].rearrange("k p n -> p k n")
        x_v = xT[:].rearrange("p (k t) -> p k t", k=KT)
        xd_v = x_d[:].rearrange("(k p) t -> p k t", p=128)
        nc.gpsimd.dma_start(
            out=cos_sb[:].rearrange("p (t n) -> p t n", t=NT),
            in_=cos_d[:].rearrange("(t p) n -> p t n", p=128),
        )
        nc.gpsimd.dma_start(
            out=sin_sb[:].rearrange("p (t n) -> p t n", t=NT),
            in_=sin_d[:].rearrange("(t p) n -> p t n", p=128),
        )
        # W + warmup x slices (tiles 0/1) first so the k-outer warmup is
        # PE-bound, then the rest of x (needed no earlier than tile 2's s1).
        for kk in range(KT):
            sl = slice(kk, kk + 1)
            nc.scalar.dma_start(out=w_v[:, sl], in_=wd_v[:, sl])
            nc.sync.dma_start(out=x_v[:, sl, 0:256], in_=xd_v[:, sl, 0:256])
        for kk in range(KT):
            sl = slice(kk, kk + 1)
            nc.sync.dma_start(out=x_v[:, sl, 256:SH], in_=xd_v[:, sl, 256:SH])
        # zero the A halves once (block-diag zeros); memset full tiles to keep
        # the sim's write-coverage tracking happy for the merged readback APs.
        for tl in bd_bufs:
            nc.gpsimd.memset(tl[:], 0.0)

        # one shared PSUM pool: 8 x [128,512] = all 8 banks, with explicit
        # slot groups (ps0-3 = group 0, ps4-7 = group 1). produce(i) runs its
        # fused projection in group i%2; consume(i)'s block-diag q matmuls
        # use the other group so slot reuse never waits on an in-flight s1.
        ps_pool = P(tc.tile_pool(name="ps", bufs=1, space="PSUM"))
        bq_pool = P(tc.tile_pool(name="bq", bufs=3))
        bqr_pool = P(tc.tile_pool(name="bqr", bufs=3))
        tmp_pool = P(tc.tile_pool(name="tmp", bufs=3))
        small_pool = P(tc.tile_pool(name="small", bufs=4))
        out_pool = P(tc.tile_pool(name="outs", bufs=3))
        dram_pool = P(tc.tile_pool(name="scr", bufs=3, space="DRAM"))

        state = {}

        def alloc_ps(nm, group):
            # 4 chunks covering the fused 2016-col projection:
            # [0:480 | 480:992 | 992:1504 | 1504:2016] mapped to 4 tiles.
            return [
                ps_pool.tile(
                    [128, 512],
                    dt.float32,
                    tag=f"ps{group * 4 + j}",
                    name=f"{nm}{j}",
                )
                for j in range(4)
            ]

        def s1_mm(it, ps, kk, start, stop):
            t0 = it * 128
            lh = xT[:, kk * SH + t0 : kk * SH + t0 + 128]
            wb = kk * NOUT
            nc.tensor.matmul(
                ps[0][:, 0:480], lh, w_sb[:, wb : wb + 480], start=start, stop=stop
            )
            for c in range(3):
                nc.tensor.matmul(
                    ps[c + 1][:],
                    lh,
                    w_sb[:, wb + 480 + c * 512 : wb + 480 + (c + 1) * 512],
                    start=start,
                    stop=stop,
                )

        def produce_post(it, ps):
            """evictions, RoPE, bounce write + scatter, k/v for tile `it`."""
            t0 = it * 128
            bd = bd_bufs[it % 3]

            # ---- PSUM evictions, spread over ACT + DVE ----
            # ps[0] holds [A'(192) | ak(16) | av(16) | bk(128) | bv(128)]
            # ps[1..3] hold B_q (1536)
            ak_sb = small_pool.tile([128, 16], dt.bfloat16, tag="ak_sb")
            av_sb = small_pool.tile([128, 16], dt.bfloat16, tag="av_sb")
            bv_sb = small_pool.tile([128, 128], dt.bfloat16, tag="bv_sb")
            # bq_sb holds raw [B_q (12x128) | B_k (128)] = 13 rank slices
            bq_sb = bq_pool.tile([128, 1664], dt.bfloat16, tag="bq_sb")
            # bqr holds roped [B_q|B_k] (0:1664) and A' (1664:1856) so the
            # DRAM bounce is a single DMA
            bqr = bqr_pool.tile([128, 1856], dt.bfloat16, tag="bqr_t")
            nc.scalar.copy(bq_sb[:, 0:512], ps[1][:])
            nc.scalar.copy(bq_sb[:, 1024:1536], ps[3][:])
            nc.vector.tensor_copy(bq_sb[:, 512:1024], ps[2][:])
            nc.vector.tensor_copy(bq_sb[:, 1536:1664], ps[0][:, 224:352])
            nc.scalar.copy(bqr[:, 1664:1856], ps[0][:, 0:192])
            nc.scalar.copy(ak_sb[:], ps[0][:, 192:208])
            nc.scalar.copy(av_sb[:], ps[0][:, 208:224])
            nc.scalar.copy(bv_sb[:], ps[0][:, 352:480])

            # ---- RoPE on [B_q|B_k] as 13 rank slices (DVE, 4 wide ops) ----
            # t = in*[cos,cos]; u = in*[+sin,-sin];
            # out_lo = t_lo + u_hi; out_hi = t_hi + u_lo
            t_a = tmp_pool.tile([128, 1664], dt.bfloat16, tag="t_a")
            t_b = tmp_pool.tile([128, 1664], dt.bfloat16, tag="t_b")
            R13 = RQ + 1
            cos_t = (
                cos_sb[:, it * 128 : (it + 1) * 128]
                .unsqueeze(1)
                .broadcast_to([128, R13, 128])
            )
            sin_t = (
                sin_sb[:, it * 128 : (it + 1) * 128]
                .unsqueeze(1)
                .broadcast_to([128, R13, 128])
            )
            bqv = bq_sb[:].rearrange("p (r c) -> p r c", r=R13)
            nc.vector.tensor_mul(
                t_a[:].rearrange("p (r c) -> p r c", r=R13), bqv, cos_t
            )
            nc.gpsimd.tensor_mul(
                t_b[:].rearrange("p (r c) -> p r c", r=R13), bqv, sin_t
            )
            tav = t_a[:].rearrange("p (r two d) -> p r two d", r=R13, two=2)
            tbv = t_b[:].rearrange("p (r two d) -> p r two d", r=R13, two=2)
            bqrv = bqr[:, 0:1664].rearrange(
                "p (r two d) -> p r two d", r=R13, two=2
            )
            nc.vector.tensor_add(bqrv[:, :, 0], tav[:, :, 0], tbv[:, :, 1])
            nc.vector.tensor_add(bqrv[:, :, 1], tav[:, :, 1], tbv[:, :, 0])
            bkr_sb = bqr[:, 1536:1664]

            # ---- bounce bqr to DRAM, then scatter back into bd ----
            scr = dram_pool.tile([128, 1856], dt.bfloat16, tag="scr_b")
            nc.sync.dma_start(out=scr[:], in_=bqr[:])

            # ---- k, v (DVE tensor_mul) + output DMAs ----
            ksb = out_pool.tile([128, 2048], dt.bfloat16, tag="ksb")
            vsb = out_pool.tile([128, 2048], dt.bfloat16, tag="vsb")
            nc.gpsimd.tensor_mul(
                ksb[:].rearrange("p (h d) -> p h d", h=NH),
                bkr_sb.unsqueeze(1).broadcast_to([128, NH, 128]),
                ak_sb[:].unsqueeze(2).broadcast_to([128, NH, 128]),
            )
            nc.vector.tensor_mul(
                vsb[:].rearrange("p (h d) -> p h d", h=NH),
                bv_sb[:].unsqueeze(1).broadcast_to([128, NH, 128]),
                av_sb[:].unsqueeze(2).broadcast_to([128, NH, 128]),
            )
            nc.gpsimd.dma_start(out=k_d[t0 : t0 + 128, :], in_=ksb[:])
            nc.scalar.dma_start(out=v_d[t0 : t0 + 128, :], in_=vsb[:])

            # scatter readbacks: B rhs into bd[:, 0:2048], A lhsT (block-diag)
            # into bd[:, 2048:4096]; 16 small DMAs spread over 4 engines.
            sa_v = scr[:, 1664:1856].rearrange(
                "(g t) (r h) -> t r g h", t=8, r=RQ
            )
            sb_v = scr[:, 0:1536].rearrange("(g t) (r d) -> t r g d", t=8, r=RQ)
            l_v = bd[0:96, 2048:4096].rearrange(
                "(t r) (g c) -> t r g c", t=8, g=16
            )
            d_v = bd[0:96, 0:2048].rearrange("(t r) (g d) -> t r g d", t=8, g=16)
            engs = (nc.gpsimd, nc.sync, nc.scalar)
            for t in range(8):
                engs[t % 3].dma_start(
                    out=l_v[t][:, :, t * 16 : (t + 1) * 16], in_=sa_v[t]
                )
                engs[(t + 1) % 3].dma_start(out=d_v[t], in_=sb_v[t])

            state[it] = bd

        def consume(it, group):
            """q contraction + output DMA for tile `it`."""
            t0 = it * 128
            bd = state.pop(it)

            qsb = out_pool.tile([128, 2048], dt.bfloat16, tag="qsb")
            for gq in range(4):
                qp = ps_pool.tile(
                    [128, 512],
                    dt.float32,
                    tag=f"ps{group * 4 + gq}",
                    name=f"qp{it}_{gq}",
                )
                for j in range(4):
                    g = gq * 4 + j
                    nc.tensor.matmul(
                        qp[:, j * 128 : (j + 1) * 128],
                        bd[0:96, 2048 + g * 128 : 2048 + (g + 1) * 128],
                        bd[0:96, g * 128 : (g + 1) * 128],
                        start=True,
                        stop=True,
                    )
                if gq % 2 == 0:
                    nc.vector.tensor_copy(
                        qsb[:, gq * 512 : (gq + 1) * 512], qp[:]
                    )
                else:
                    nc.scalar.copy(qsb[:, gq * 512 : (gq + 1) * 512], qp[:])

            # dense on-chip-layout store; host reorders (t,h)(g,d)->(g,t)(h,d)
            nc.sync.dma_start(out=q_d[t0 : t0 + 128, :], in_=qsb[:])

        # ---- schedule ----
        # warmup: tiles 0 and 1 k-outer, PSUM-resident (8 banks), so the PE
        # starts as soon as the first W/x k-chunk lands.
        ps0 = alloc_ps("w0_", 0)
        ps1 = alloc_ps("w1_", 1)
        for kk in range(KT):
            s1_mm(0, ps0, kk, start=(kk == 0), stop=(kk == KT - 1))
            s1_mm(1, ps1, kk, start=(kk == 0), stop=(kk == KT - 1))
        produce_post(0, ps0)
        produce_post(1, ps1)

        def produce(it, hooks=None):
            ps = alloc_ps(f"p{it}_", it % 2)
            for kk in range(KT):
                if hooks and kk in hooks:
                    hooks[kk]()
                s1_mm(it, ps, kk, start=(kk == 0), stop=(kk == KT - 1))
            produce_post(it, ps)

        # steady state: consume(i) is emitted mid-way through produce(i+2)'s
        # k-loop so its instructions sit ahead of produce(i+2)'s
        # post-processing in every engine stream (engine queues are serial),
        # and its chain gets ~half a tile of slack before the BD matmuls.
        produce(2, {10: lambda: consume(0, 1)})
        produce(3, {10: lambda: consume(1, 0)})
        produce(4, {10: lambda: consume(2, 1)})
        produce(5, {10: lambda: consume(3, 0)})
        produce(6, {10: lambda: consume(4, 1)})
        produce(7, {4: lambda: consume(5, 0), 12: lambda: consume(6, 0)})
        consume(7, 1)


def build_program():
    import concourse.tile as tile

    nc, tensors = make_nc()
    with tile.TileContext(nc) as tc:
        build_body(nc, tc, tensors)
    nc.compile()
    return nc


def _get_program():
    if "nc" not in _CACHE:
        _CACHE["nc"] = build_program()
    return _CACHE["nc"]


def make_in_maps(x, W_A_q, W_B_q, W_A_k, W_B_k, W_A_v, W_B_v):
    """Shard + preprocess full inputs into per-core input maps."""
    x = np.asarray(x)
    B, S, Hh = x.shape
    x2 = np.ascontiguousarray(x.reshape(B * S, Hh))

    # fold the 1/RQ scale and the (h,r)->(r,h) column reorder into W_A_q
    WAq = np.asarray(W_A_q).reshape(Hh, NH, RQ).transpose(0, 2, 1).reshape(
        Hh, NH * RQ
    ) / np.float32(RQ)
    Wall = np.concatenate(
        [
            WAq,
            np.asarray(W_A_k),
            np.asarray(W_A_v),
            np.asarray(W_B_k),
            np.asarray(W_B_v),
            np.asarray(W_B_q),
        ],
        axis=1,
    )
    assert Wall.shape == (Hh, NOUT)
    Wt = np.ascontiguousarray(Wall.reshape(KT, 128, NOUT)).astype(BF16)

    inv = 1.0 / (10000.0 ** (np.arange(0, HD, 2, dtype=np.float32) / HD))
    ang = np.arange(S, dtype=np.float32)[:, None] * inv[None, :]
    c, s = np.cos(ang), np.sin(ang)
    cos2 = np.ascontiguousarray(np.concatenate([c, c], axis=1)).astype(BF16)
    sinsw = np.ascontiguousarray(np.concatenate([s, -s], axis=1)).astype(BF16)

    in_maps = []
    for i in range(8):
        tok0 = i * SH
        pos = np.arange(tok0, tok0 + SH) % S
        in_maps.append(
            {
                # pre-transposed (hidden, tokens) so on-chip loads are plain
                "x": np.ascontiguousarray(x2[tok0 : tok0 + SH].T).astype(BF16),
                "w": Wt,
                "cos2": np.ascontiguousarray(cos2[pos]),
                "sinsw": np.ascontiguousarray(sinsw[pos]),
            }
        )
    return in_maps, (B, S)


def assemble_outputs(results, B, S):
    # q rows are (tile, t, h) with token = tile*128 + g*8 + t, cols (g, d)
    qs = []
    for i in range(8):
        a = results[i]["q"].astype(np.float32)
        a = a.reshape(NT, 8, 16, 16, 128).transpose(0, 3, 1, 2, 4)
        qs.append(a.reshape(SH, NH, HD))
    q = np.concatenate(qs, axis=0).reshape(B, S, NH, HD)
    k = np.concatenate(
        [results[i]["k"].astype(np.float32) for i in range(8)], axis=0
    ).reshape(B, S, NH, HD)
    v = np.concatenate(
        [results[i]["v"].astype(np.float32) for i in range(8)], axis=0
    ).reshape(B, S, NH, HD)
    return q, k, v


def kernel(x, W_A_q, W_B_q, W_A_k, W_B_k, W_A_v, W_B_v):
    from concourse.bass_utils import run_bass_kernel_spmd

    nc = _get_program()
    in_maps, (B, S) = make_in_maps(x, W_A_q, W_B_q, W_A_k, W_B_k, W_A_v, W_B_v)
    res = run_bass_kernel_spmd(nc, in_maps, list(range(8))).results
    return assemble_outputs(res, B, S)
